# revision 49
# baseline (speedup 1.0000x reference)
"""MetaPathEncoder (4x GraphConv + mean fusion) as a Bass/Tile SPMD kernel on 8 TRN2 cores.

Strategy (1D dst-node sharding, all 4 metapaths per core):
  - Each core owns 1250 output rows (10000/8). Edges are bucketed on host by
    (core, path, 128-row dst tile) and source-deduplicated per bucket.
  - GraphConv norm factorization: deg_out(src)^-1/2 is pre-multiplied into a
    per-path copy of the features (bf16, in HBM); deg_in(dst)^-1/2 * 1/4 is
    applied per dst row by the ACT engine when copying the SpMM result out of
    PSUM. The scatter matrix S[slot, dst_local] therefore holds small-int
    edge counts, shipped as uint8 (half the bytes) and upconverted to bf16 on
    the otherwise-idle DVE.
  - On device, per (tile, path): dma_gather the unique source rows (bf16) of
    feat_p from HBM. Gathers are issued round-robin on SWDGE queues 0-3 so
    the four Q7 core pairs generate DMA descriptors concurrently; deep tile
    pools keep several gathers in flight.
  - Segment-sum via PE matmuls accumulating in fp32 PSUM:
    h[dst, :] = sum_b S_b.T @ X_b; ACT copies h out of PSUM with the
    per-dst-row scale; PE transposes h (identity matmul) to fi-on-partitions;
    16 accumulating matmuls apply the four 512x512 weights:
    out = sum_p h_p @ W_p + mean(b). The [1250, 512] fp32 shard is DMA'd out
    and the host concatenates the 8 shards.
"""
import os
import sys

for _p in ("/opt/trn_rl_repo",):
    if _p not in sys.path:
        sys.path.insert(0, _p)

import numpy as np
import ml_dtypes

import concourse.bass as bass
import concourse.tile as tile
from concourse import bacc, mybir
from concourse.bass_utils import run_bass_kernel_spmd
BF16 = ml_dtypes.bfloat16
F8E4 = ml_dtypes.float8_e4m3

N_NODES = 10000
N_PATHS = 4
IN_DIM = 512
OUT_DIM = 512
NCORES = 8
ROWS_PER_CORE = N_NODES // NCORES  # 1250
NTILES = (ROWS_PER_CORE + 127) // 128  # 10 (last tile has 98 rows)
NCALLS = NTILES * N_PATHS  # 40 gather calls per core

_program_cache: dict[tuple, object] = {}


def _build_program(Bc: tuple):
    """Build the SPMD Bass program; Bc[call] = gather blocks for call (t*4+p)."""
    if Bc in _program_cache:
        return _program_cache[Bc]

    TI = sum(Bc) * 8    # idx cols (int16, wrapped 16x, replicated 8x)
    TS = sum(Bc) * 128  # S cols (uint8)

    dt = mybir.dt
    nc = bacc.Bacc(
        "TRN2",
        target_bir_lowering=False,
        debug=False,
        num_devices=NCORES,
        num_swdge_queues=4,
    )

    featd = [
        nc.dram_tensor(f"feat{p}", [N_NODES, IN_DIM], dt.bfloat16, kind="ExternalInput").ap()
        for p in range(N_PATHS)
    ]
    idxd = nc.dram_tensor("idx", [128, TI], dt.int16, kind="ExternalInput").ap()
    sd = nc.dram_tensor("smat", [128, TS], dt.uint8, kind="ExternalInput").ap()
    wd = nc.dram_tensor("w", [128, 16 * OUT_DIM], dt.bfloat16, kind="ExternalInput").ap()
    bmd = nc.dram_tensor("bm", [128, OUT_DIM], dt.float32, kind="ExternalInput").ap()
    bsd = nc.dram_tensor("bscale", [128, NCALLS], dt.float32, kind="ExternalInput").ap()
    identd = nc.dram_tensor("identity", [128, 128], dt.bfloat16, kind="ExternalInput").ap()
    outd = nc.dram_tensor("out", [ROWS_PER_CORE, OUT_DIM], dt.float32, kind="ExternalOutput").ap()

    with tile.TileContext(nc) as tc:
        with (
            tc.tile_pool(name="const", bufs=1) as cpool,
            tc.tile_pool(name="g", bufs=7) as gpool,
            tc.tile_pool(name="s8", bufs=8) as s8pool,
            tc.tile_pool(name="sb", bufs=4) as sbpool,
            tc.tile_pool(name="hsb", bufs=3) as hsb_pool,
            tc.tile_pool(name="htsb", bufs=3) as htsb_pool,
            tc.tile_pool(name="osb", bufs=2) as osb_pool,
            tc.tile_pool(name="hps", bufs=3, space="PSUM") as hps_pool,
            tc.tile_pool(name="htps", bufs=2, space="PSUM") as htps_pool,
            tc.tile_pool(name="ops", bufs=2, space="PSUM") as ops_pool,
        ):
            idx_sb = cpool.tile([128, TI], dt.int16)
            nc.sync.dma_start(idx_sb[:], idxd[:])
            w_sb = cpool.tile([128, 16 * OUT_DIM], dt.bfloat16)
            nc.sync.dma_start(w_sb[:], wd[:])
            bm_sb = cpool.tile([128, OUT_DIM], dt.float32)
            nc.sync.dma_start(bm_sb[:], bmd[:])
            bs_sb = cpool.tile([128, NCALLS], dt.float32)
            nc.sync.dma_start(bs_sb[:], bsd[:])
            ident = cpool.tile([128, 128], dt.bfloat16)
            nc.sync.dma_start(ident[:], identd[:])

            off_i = [0]
            off_s = [0]
            for b in Bc:
                off_i.append(off_i[-1] + b * 8)
                off_s.append(off_s[-1] + b * 128)

            for t in range(NTILES):
                out_ps = ops_pool.tile([128, OUT_DIM], dt.float32)
                for p in range(N_PATHS):
                    call = t * N_PATHS + p
                    B = Bc[call]
                    g = gpool.tile([128, B, IN_DIM], dt.bfloat16)
                    nc.gpsimd.dma_gather(
                        g[:],
                        featd[p][:],
                        idx_sb[:, off_i[call] : off_i[call + 1]],
                        B * 128,
                        B * 128,
                        IN_DIM,
                        single_packet=False,
                        queue_num=0 if os.environ.get("KQ") == "0" else p,
                    )
                    s8 = s8pool.tile([128, B * 128], dt.uint8)
                    nc.sync.dma_start(s8[:], sd[:, off_s[call] : off_s[call + 1]])
                    S = sbpool.tile([128, B * 128], dt.bfloat16)
                    nc.scalar.copy(S[:], s8[:])
                    hp = hps_pool.tile([128, IN_DIM], dt.float32)
                    for bb in range(B):
                        nc.tensor.matmul(
                            hp[:],
                            S[:, bb * 128 : (bb + 1) * 128],
                            g[:, bb, :],
                            start=(bb == 0),
                            stop=(bb == B - 1),
                        )
                    hs = hsb_pool.tile([128, IN_DIM], dt.bfloat16)
                    nc.scalar.mul(hs[:], hp[:], bs_sb[:, call : call + 1])
                    htp = htps_pool.tile([128, IN_DIM], dt.bfloat16)
                    for cc in range(4):
                        nc.tensor.transpose(
                            htp[:, cc * 128 : (cc + 1) * 128],
                            hs[:, cc * 128 : (cc + 1) * 128],
                            ident[:],
                        )
                    hts = htsb_pool.tile([128, IN_DIM], dt.bfloat16)
                    nc.vector.tensor_copy(hts[:], htp[:])
                    for cc in range(4):
                        nc.tensor.matmul(
                            out_ps[:],
                            hts[:, cc * 128 : (cc + 1) * 128],
                            w_sb[:, (p * 4 + cc) * OUT_DIM : (p * 4 + cc + 1) * OUT_DIM],
                            start=(p == 0 and cc == 0),
                            stop=(p == N_PATHS - 1 and cc == 3),
                        )
                os_ = osb_pool.tile([128, OUT_DIM], dt.float32)
                nc.vector.tensor_add(os_[:], out_ps[:], bm_sb[:])
                rows = min(128, ROWS_PER_CORE - t * 128)
                nc.sync.dma_start(outd[t * 128 : t * 128 + rows, :], os_[:rows, :])

    nc.compile()
    _program_cache[Bc] = nc
    return nc


def _prep_host(feat, src, dst, W, b):
    """Host-side bucketing, dedup, factored norms, and uint8 S materialization.

    Returns (Bc tuple, shared dict, per-core dicts)."""
    src = np.asarray(src).astype(np.int64)
    dst = np.asarray(dst).astype(np.int64)
    feat = np.asarray(feat, dtype=np.float32)
    W = np.asarray(W, dtype=np.float32)
    b = np.asarray(b, dtype=np.float32)

    # weights laid out [fi_local(128), p*4+chunk, fo] for direct SBUF residence
    Wt = np.empty((128, 16, OUT_DIM), dtype=BF16)
    for p in range(N_PATHS):
        for c in range(4):
            Wt[:, p * 4 + c, :] = W[p, c * 128 : (c + 1) * 128, :].astype(BF16)
    Wt = np.ascontiguousarray(Wt.reshape(128, 16 * OUT_DIM))

    bmean = b.mean(0).astype(np.float32)
    bm_bcast = np.ascontiguousarray(np.broadcast_to(bmean, (128, OUT_DIM)))

    # factored norms: feat_p = feat * deg_out_p^-1/2 (bf16 in HBM);
    # bscale[dst] = deg_in_p(dst)^-1/2 * 1/4 applied post-SpMM on ACT
    feats = {}
    deg_ins = []
    sorted_data = []
    for p in range(N_PATHS):
        s, d = src[p], dst[p]
        deg_out = np.maximum(np.bincount(s, minlength=N_NODES), 1).astype(np.float64)
        deg_in = np.maximum(np.bincount(d, minlength=N_NODES), 1).astype(np.float64)
        feats[f"feat{p}"] = (feat * (deg_out**-0.5)[:, None]).astype(BF16)
        deg_ins.append(deg_in)
        order = np.argsort(d, kind="stable")
        sorted_data.append((s[order], d[order]))

    bounds = []
    for c in range(NCORES):
        base = c * ROWS_PER_CORE
        for t in range(NTILES):
            lo = base + t * 128
            hi = base + min((t + 1) * 128, ROWS_PER_CORE)
            bounds.append((lo, hi))
    los = np.array([lo for lo, _ in bounds])
    his = np.array([hi for _, hi in bounds])

    ranges = []
    for p in range(N_PATHS):
        ds = sorted_data[p][1]
        a = np.searchsorted(ds, los, side="left")
        e = np.searchsorted(ds, his, side="left")
        ranges.append((a, e))

    # dedup per (core, path, tile); Bc[call] = max over cores
    buckets = {}  # (c, call) -> (uniq_idx, S_u8 [U, 128])
    Bc = np.zeros(NCALLS, dtype=np.int64)
    for c in range(NCORES):
        for t in range(NTILES):
            lo = c * ROWS_PER_CORE + t * 128
            for p in range(N_PATHS):
                call = t * N_PATHS + p
                a, e = ranges[p][0][c * NTILES + t], ranges[p][1][c * NTILES + t]
                ss = sorted_data[p][0][a:e]
                dl = (sorted_data[p][1][a:e] - lo).astype(np.int64)
                uniq, inv = np.unique(ss, return_inverse=True)
                U = len(uniq)
                S = np.zeros((U, 128), dtype=np.int64)
                np.add.at(S, (inv, dl), 1)
                assert S.max() < 256
                buckets[(c, call)] = (uniq, S.astype(np.uint8))
                Bc[call] = max(Bc[call], (U + 127) // 128)
    Bc = np.maximum(Bc, 1)

    off_i = np.concatenate([[0], np.cumsum(Bc * 8)])
    off_s = np.concatenate([[0], np.cumsum(Bc * 128)])
    TI, TS = int(off_i[-1]), int(off_s[-1])

    per_core = []
    for c in range(NCORES):
        idxw = np.zeros((128, TI), dtype=np.int16)
        s_cols = np.zeros((128, TS), dtype=np.uint8)
        bsc = np.zeros((128, NCALLS), dtype=np.float32)
        for call in range(NCALLS):
            B = int(Bc[call])
            t, p = call // N_PATHS, call % N_PATHS
            uniq, S = buckets[(c, call)]
            U = len(uniq)
            idx_pad = np.zeros(B * 128, dtype=np.int16)
            idx_pad[:U] = uniq
            # dma_gather wrapped index layout: position j -> [j%16, j//16],
            # replicated across the 8 groups of 16 partitions
            w16 = idx_pad.reshape(B * 8, 16).T  # [16, B*8]
            idxw[:, off_i[call] : off_i[call + 1]] = np.tile(w16, (8, 1))
            S_pad = np.zeros((B * 128, 128), dtype=np.uint8)
            S_pad[:U] = S
            s_cols[:, off_s[call] : off_s[call + 1]] = (
                S_pad.reshape(B, 128, 128).transpose(1, 0, 2).reshape(128, B * 128)
            )
            lo = c * ROWS_PER_CORE + t * 128
            rows = min(128, ROWS_PER_CORE - t * 128)
            bsc[:rows, call] = (deg_ins[p][lo : lo + rows] ** -0.5) * 0.25
        per_core.append({"idx": idxw, "smat": s_cols, "bscale": bsc})

    shared = {
        **feats,
        "w": Wt,
        "bm": bm_bcast,
        "identity": np.eye(128, dtype=BF16),
    }
    return tuple(int(x) for x in Bc), shared, per_core


def kernel(feat, src, dst, W, b):
    Bc, shared, per_core = _prep_host(feat, src, dst, W, b)
    nc = _build_program(Bc)
    in_maps = [{**shared, **pc} for pc in per_core]
    res = run_bass_kernel_spmd(nc, in_maps, list(range(NCORES)))
    out = np.concatenate([res.results[c]["out"] for c in range(NCORES)], axis=0)
    return out.astype(np.float32)


if __name__ == "__main__":
    rng = np.random.default_rng(0)
    feat = rng.standard_normal((N_NODES, IN_DIM), dtype=np.float32)
    src = rng.integers(0, N_NODES, (N_PATHS, 160000)).astype(np.int64)
    dst = rng.integers(0, N_NODES, (N_PATHS, 160000)).astype(np.int64)
    W = (rng.standard_normal((N_PATHS, IN_DIM, OUT_DIM), dtype=np.float32) / np.sqrt(IN_DIM)).astype(np.float32)
    b = np.zeros((N_PATHS, OUT_DIM), np.float32)
    out = kernel(feat=feat, src=src, dst=dst, W=W, b=b)
    print("kernel ran, out shape", out.shape, out.dtype)
